# revision 12
# baseline (speedup 1.0000x reference)
"""LocalContrastEnhancement v4: decimated-horizontal-scan Bass kernel, 8 trn2 cores.

out = (x - mean) / (sqrt(max(var, 1e-6)) + 1e-6), 15x15 zero-padded box.

Sharding: pure data parallel, 1 image (3,1024,1024) per NeuronCore.

v4 halves the DVE scan cost (the v1-v3 bottleneck: 2 scans x 2.17ns/elem
x 1031 cols) by folding columns into pairs and scanning a 7-PAIR window
(14-col sums at stride 2), then adding the 15th column inside the PE
band matmuls as an extra accumulated matmul over a compact fp16
even/odd column tile. Even and odd output columns become two 512-wide
phases sharing the same scan output.

Per stripe (K<=128 input rows, M=114 out rows):
  ACT: xe/xo = fp16(x[even/odd cols]), sqe/sqo = fp16((x-.5)^2[e/o])
       (strided reads, compact outputs -> every matmul operand is
       contiguous), s1sq = (S1~)^2 and rsqrt over merged 1024-wide psum.
  DVE: fold yx=xe+xo, ysq=sqe+sqo (fp16 2x), two 519-long scans
       (7-pair windows, centered via the scan initial), two 512-wide
       STTs writing interleaved columns of the group output tile.
  PE (12 matmuls/stripe, all 512-free fp16):
       PD[:,e] = -band*o1 - band*xo[corr] + iden*xe   (odd mirrored)
       P2[:,e] = 225*band*o2 + 225*band*sqo[corr] - I*s1sq
  Vertical pad rows are corrected via per-row constants folded into the
  ACT biases / STT scalar (raw-pad algebra: see corr vectors).

DMA queue rule (measured): a transfer lands on the largest divisor
<=16 of its PARTITION count many queues. 114-partition stores hit only
6 queues; so stores are split at partition 112 (=16*7 -> all 16
queues), and input loads likewise avoid non-16-divisible row counts.
"""

import numpy as np

C, H, W = 3, 1024, 1024
NCORES = 8
KS = 15
HALF = 7
XP = 8  # left pad cols in the xt row buffer
BX = XP + W + 8  # 1040
NP = 520  # compact even/odd tile length (image cols -8..1031)
YB = 527  # fold buffer: 7 left pad pairs + 520
NSC = 519  # scan output length; o1[s] = 14-col sum for out col pair j=s-7
MSTR = 114  # out rows per stripe (uniform; bottom stripe rows >=1024 trimmed)
NSTR = 9  # stripes per channel
GRP = 3  # stripes batched per output store

_CACHE = {}


def _stripes():
    """(r_in0, K, variant) per stripe; r_out0 = 114*t. variant: 0 top, 1 bottom, 2 interior."""
    out = []
    for t in range(NSTR):
        r_out0 = MSTR * t
        r_in0 = max(r_out0 - HALF, 0)
        r_in1 = min(r_out0 + MSTR - 1 + HALF, H - 1)
        k = r_in1 - r_in0 + 1
        v = 0 if t == 0 else (1 if t == NSTR - 1 else 2)
        out.append((r_in0, k, v))
    return out


def _const_mats():
    band = np.zeros((128, MSTR), dtype=np.float32)
    iden = np.zeros((128, MSTR), dtype=np.float32)
    for m in range(MSTR):
        band[m : m + KS, m] = 1.0
        iden[m + HALF, m] = 225.0
    band_top = np.zeros_like(band)
    band_top[0:121, :] = band[7:128, :]
    iden_top = np.zeros_like(iden)
    iden_top[0:121, :] = iden[7:128, :]
    negi = np.zeros((128, MSTR), dtype=np.float32)
    for m in range(MSTR):
        negi[m, m] = -1.0
    bands = np.stack(
        [-band, 225.0 * band, -band_top, 225.0 * band_top, negi], axis=1
    )  # [128, 5, 114] fp16
    idens = np.stack([iden, iden_top], axis=1).astype(np.float16)

    # Per-out-row vertical pad corrections (raw-pad algebra):
    #   s1sq bias   = -7.5 - 7n      (S1~true = -PD_ph1 - 7.5 - 7n)
    #   rsqrt bias  = 843.75 n       (225*S2~true = P2 + 843.75n)
    #   stt scalar  = 7n - 105       (num = PD + 7n - 105)
    m_idx = np.arange(128)
    n_top = np.maximum(0, HALF - m_idx).astype(np.float32)
    n_bot = np.maximum(0, m_idx - 104).astype(np.float32)
    corr = np.zeros((128, 3, 3), dtype=np.float32)
    for v, n in ((0, n_top), (1, n_bot), (2, np.zeros(128, np.float32))):
        corr[:, v, 0] = -7.5 - 7.0 * n
        corr[:, v, 1] = 843.75 * n
        corr[:, v, 2] = 7.0 * n - 105.0
    return bands.astype(np.float16), idens, corr


def _build_nc():
    import concourse.bass as bass
    import concourse.bacc as bacc
    import concourse.tile as tile
    from concourse import mybir
    import bass_rust as _bass_rust
    from concourse.hw_specs import get_activation_tables

    f32 = mybir.dt.float32
    fp16 = mybir.dt.float16
    Alu = mybir.AluOpType
    Act = mybir.ActivationFunctionType

    class _LceBacc(bacc.Bacc):
        """Pin act-table selection to the set holding Square+Copy+AbsRsqrt."""

        def insert_act_table_loads(self):
            tables = [
                (name, funcs if name == "abs_reciprocal_sqrt_and_small" else set())
                for name, funcs in get_activation_tables(self.m.arch).items()
            ]
            _bass_rust.insert_act_table_loads(self, tables)

    nc = _LceBacc(trn_type="TRN2", target_bir_lowering=False)
    x_d = nc.dram_tensor("x", [C, H, W], f32, kind="ExternalInput")
    bands_d = nc.dram_tensor("bands", [128, 5, MSTR], fp16, kind="ExternalInput")
    iden_d = nc.dram_tensor("iden", [128, 2, MSTR], fp16, kind="ExternalInput")
    corr_d = nc.dram_tensor("corr", [128, 3, 3], f32, kind="ExternalInput")
    y_d = nc.dram_tensor("y", [C, H, W], f32, kind="ExternalOutput")

    stripes = _stripes()

    from contextlib import ExitStack

    with tile.TileContext(nc) as tc, ExitStack() as ctx:
        singles = ctx.enter_context(tc.tile_pool(name="singles", bufs=1))
        io_pool = ctx.enter_context(tc.tile_pool(name="io", bufs=1))
        s1sq_p = ctx.enter_context(tc.tile_pool(name="s1sq", bufs=3))
        r_p = ctx.enter_context(tc.tile_pool(name="rts", bufs=3))
        out_p = ctx.enter_context(tc.tile_pool(name="outb", bufs=3))
        psd_p = ctx.enter_context(tc.tile_pool(name="psd", bufs=2, space="PSUM"))
        ps2_p = ctx.enter_context(tc.tile_pool(name="ps2", bufs=2, space="PSUM"))

        bands_t = singles.tile([128, 5, MSTR], fp16)
        iden_t = singles.tile([128, 2, MSTR], fp16)
        corr_t = singles.tile([128, 3, 3], f32)
        nc.sync.dma_start(out=bands_t[:, :, :], in_=bands_d[:, :, :])
        nc.sync.dma_start(out=iden_t[:, :, :], in_=iden_d[:, :, :])
        nc.sync.dma_start(out=corr_t[:, :, :], in_=corr_d[:, :, :])

        NBUF = 5
        xb = [io_pool.tile([128, BX], f32, tag=f"xb{i}", name=f"xb{i}") for i in range(NBUF)]
        xe = [io_pool.tile([128, NP], fp16, tag=f"xe{i}", name=f"xe{i}") for i in range(NBUF)]
        xo = [io_pool.tile([128, NP], fp16, tag=f"xo{i}", name=f"xo{i}") for i in range(NBUF)]
        sqe = [io_pool.tile([128, NP], fp16, tag=f"sqe{i}", name=f"sqe{i}") for i in range(NBUF)]
        sqo = [io_pool.tile([128, NP], fp16, tag=f"sqo{i}", name=f"sqo{i}") for i in range(NBUF)]
        yx = [io_pool.tile([128, YB], fp16, tag=f"yx{i}", name=f"yx{i}") for i in range(NBUF)]
        ysq = [io_pool.tile([128, YB], fp16, tag=f"ysq{i}", name=f"ysq{i}") for i in range(NBUF)]
        ob1 = [io_pool.tile([128, NSC], fp16, tag=f"ob1{i}", name=f"ob1{i}") for i in range(NBUF)]
        ob2 = [io_pool.tile([128, NSC], fp16, tag=f"ob2{i}", name=f"ob2{i}") for i in range(NBUF)]
        for i in range(NBUF):
            nc.vector.memset(xb[i][:, 0:XP], 0.0)
            nc.vector.memset(xb[i][:, XP + W : BX], 0.0)
            nc.vector.memset(yx[i][:, 0:7], 0.0)
            nc.vector.memset(ysq[i][:, 0:7], 0.5)

        neghalf = singles.tile([128, 1], f32)
        nc.vector.memset(neghalf[:, :], -0.5)
        # ACT warm-ups: absorb const-DMA / memset sync ticks outside the loop
        warm1 = singles.tile([128, 1], f32)
        warm2 = singles.tile([128, 1], f32)
        warm3 = singles.tile([128, 1], f32)
        warm4 = singles.tile([128, 1], f32)
        nc.scalar.activation(out=warm1[:, :], in_=corr_t[:, 0, 0:1], func=Act.Square)
        nc.scalar.activation(out=warm2[:, :], in_=iden_t[:, 0, 0:1], func=Act.Square)
        nc.scalar.activation(out=warm3[:, :], in_=neghalf[:, :], func=Act.Square)
        nc.scalar.activation(
            out=warm4[:, :], in_=warm3[:, :], func=Act.Abs_reciprocal_sqrt
        )

        def stage_load(idx):
            """DMA in + compact casts for stripe idx (hoisted one stripe
            ahead so the in-order ACT queue never stalls on s1sq deps)."""
            c, t = divmod(idx, NSTR)
            r_in0, K, vv = stripes[t]
            i5 = idx % NBUF
            xt = xb[i5]

            # split loads so partition counts divide by 16 (queue spread)
            if K == 128:
                nc.sync.dma_start(
                    out=xt[0:K, XP : XP + W],
                    in_=x_d[c, r_in0 : r_in0 + K, :],
                )
            else:
                nc.sync.dma_start(
                    out=xt[0:112, XP : XP + W],
                    in_=x_d[c, r_in0 : r_in0 + 112, :],
                )
                nc.sync.dma_start(
                    out=xt[112:K, XP : XP + W],
                    in_=x_d[c, r_in0 + 112 : r_in0 + K, :],
                )

            # compact fp16 even/odd tiles (strided ACT reads)
            nc.scalar.activation(
                out=xe[i5][0:K, :], in_=xt[0:K, 0:BX:2], func=Act.Copy
            )
            nc.scalar.activation(
                out=xo[i5][0:K, :], in_=xt[0:K, 1:BX:2], func=Act.Copy
            )
            nc.scalar.activation(
                out=sqe[i5][0:K, :],
                in_=xt[0:K, 0:BX:2],
                func=Act.Square,
                bias=neghalf[0:K, 0:1],
            )
            nc.scalar.activation(
                out=sqo[i5][0:K, :],
                in_=xt[0:K, 1:BX:2],
                func=Act.Square,
                bias=neghalf[0:K, 0:1],
            )

        def stage_vec(idx):
            """Pair folds + scans for stripe idx."""
            c, t = divmod(idx, NSTR)
            r_in0, K, vv = stripes[t]
            i5 = idx % NBUF
            # pair folds (fp16 2x) and 7-pair scans (centered via initial)
            nc.vector.tensor_tensor(
                out=yx[i5][0:K, 7:YB],
                in0=xe[i5][0:K, :],
                in1=xo[i5][0:K, :],
                op=Alu.add,
            )
            nc.vector.tensor_tensor(
                out=ysq[i5][0:K, 7:YB],
                in0=sqe[i5][0:K, :],
                in1=sqo[i5][0:K, :],
                op=Alu.add,
            )
            nc.vector.tensor_tensor_scan(
                out=ob1[i5][0:K, 0:NSC],
                data0=yx[i5][0:K, 7 : 7 + NSC],
                data1=yx[i5][0:K, 0:NSC],
                initial=-7.0,
                op0=Alu.add,
                op1=Alu.subtract,
            )
            nc.vector.tensor_tensor_scan(
                out=ob2[i5][0:K, 0:NSC],
                data0=ysq[i5][0:K, 7 : 7 + NSC],
                data1=ysq[i5][0:K, 0:NSC],
                initial=3.5,
                op0=Alu.add,
                op1=Alu.subtract,
            )

        tiles = {}

        def stage_mm_early(idx):
            """Phase-1 band matmuls (4) for stripe idx; allocates pd."""
            c, t = divmod(idx, NSTR)
            r_in0, K, vv = stripes[t]
            i5 = idx % NBUF
            bsel = 2 if vv == 0 else 0
            o1 = ob1[i5]
            pd = psd_p.tile([MSTR, W], f32, tag="pd", name="pd")
            tiles[idx] = {"pd": pd}
            nc.tensor.matmul(
                pd[0:MSTR, 0:512],
                bands_t[0:K, bsel, 0:MSTR],
                o1[0:K, 7 : 7 + 512],
                start=True,
                stop=False,
            )
            nc.tensor.matmul(
                pd[0:MSTR, 0:512],
                bands_t[0:K, bsel, 0:MSTR],
                xo[i5][0:K, 0:512],
                start=False,
                stop=False,
            )
            nc.tensor.matmul(
                pd[0:MSTR, 512:1024],
                bands_t[0:K, bsel, 0:MSTR],
                o1[0:K, 7 : 7 + 512],
                start=True,
                stop=False,
            )
            nc.tensor.matmul(
                pd[0:MSTR, 512:1024],
                bands_t[0:K, bsel, 0:MSTR],
                xe[i5][0:K, 8:520],
                start=False,
                stop=False,
            )

        def stage_s1sq(idx):
            """s1sq = (-PD + corr0)^2, fp16, mid-group psum read."""
            c, t = divmod(idx, NSTR)
            r_in0, K, vv = stripes[t]
            pd = tiles[idx]["pd"]
            s1sq = s1sq_p.tile([MSTR, W], fp16, tag="s1sq", name="s1sq")
            tiles[idx]["s1sq"] = s1sq
            nc.scalar.activation(
                out=s1sq[0:MSTR, :],
                in_=pd[0:MSTR, :],
                func=Act.Square,
                scale=-1.0,
                bias=corr_t[0:MSTR, vv, 0:1],
            )

        def stage_mm_late(idx):
            """Phase-2 iden matmuls (2) + P2 matmuls (6) for stripe idx."""
            c, t = divmod(idx, NSTR)
            r_in0, K, vv = stripes[t]
            i5 = idx % NBUF
            bsel = 2 if vv == 0 else 0
            isel = 1 if vv == 0 else 0
            o2 = ob2[i5]
            pd = tiles[idx]["pd"]
            s1sq = tiles[idx]["s1sq"]
            p2 = ps2_p.tile([MSTR, W], f32, tag="p2", name="p2")
            tiles[idx]["p2"] = p2
            nc.tensor.matmul(
                pd[0:MSTR, 0:512],
                iden_t[0:K, isel, 0:MSTR],
                xe[i5][0:K, 4:516],
                start=False,
                stop=True,
                skip_group_check=True,
            )
            nc.tensor.matmul(
                pd[0:MSTR, 512:1024],
                iden_t[0:K, isel, 0:MSTR],
                xo[i5][0:K, 4:516],
                start=False,
                stop=True,
                skip_group_check=True,
            )
            nc.tensor.matmul(
                p2[0:MSTR, 0:512],
                bands_t[0:K, bsel + 1, 0:MSTR],
                o2[0:K, 7 : 7 + 512],
                start=True,
                stop=False,
            )
            nc.tensor.matmul(
                p2[0:MSTR, 0:512],
                bands_t[0:K, bsel + 1, 0:MSTR],
                sqo[i5][0:K, 0:512],
                start=False,
                stop=False,
            )
            nc.tensor.matmul(
                p2[0:MSTR, 0:512],
                bands_t[0:MSTR, 4, 0:MSTR],
                s1sq[0:MSTR, 0:512],
                start=False,
                stop=True,
            )
            nc.tensor.matmul(
                p2[0:MSTR, 512:1024],
                bands_t[0:K, bsel + 1, 0:MSTR],
                o2[0:K, 7 : 7 + 512],
                start=True,
                stop=False,
            )
            nc.tensor.matmul(
                p2[0:MSTR, 512:1024],
                bands_t[0:K, bsel + 1, 0:MSTR],
                sqe[i5][0:K, 8:520],
                start=False,
                stop=False,
            )
            nc.tensor.matmul(
                p2[0:MSTR, 512:1024],
                bands_t[0:MSTR, 4, 0:MSTR],
                s1sq[0:MSTR, 512:1024],
                start=False,
                stop=True,
            )

        def stage_rsq(idx):
            """rsqrt for stripe idx (deps one iteration stale)."""
            c, t = divmod(idx, NSTR)
            r_in0, K, vv = stripes[t]
            p2 = tiles[idx]["p2"]
            rts = r_p.tile([MSTR, W], f32, tag="rts", name="rts")
            tiles[idx]["rts"] = rts
            nc.scalar.activation(
                out=rts[0:MSTR, :],
                in_=p2[0:MSTR, :],
                func=Act.Abs_reciprocal_sqrt,
                bias=corr_t[0:MSTR, vv, 1:2],
            )

        def stage_fin(idx):
            """Final combine + stores for stripe idx (no ACT ops here: the
            store issue must not sit mid-stream on the ACT sequencer)."""
            c, t = divmod(idx, NSTR)
            r_in0, K, vv = stripes[t]
            r_out0 = MSTR * t
            pd = tiles[idx]["pd"]
            rts = tiles[idx]["rts"]
            # out = (PD + corr2) * R in half-layout (cols [even|odd]);
            # python de-interleaves during unshard
            outb = out_p.tile([MSTR, W], f32, tag="outb", name="outb")
            nc.vector.scalar_tensor_tensor(
                out=outb[0:MSTR, 0:W],
                in0=pd[0:MSTR, 0:W],
                scalar=corr_t[0:MSTR, vv, 2:3],
                in1=rts[0:MSTR, 0:W],
                op0=Alu.add,
                op1=Alu.mult,
            )
            # stores: 112 partitions -> 16 queues; 2-row remainder apart
            nc.sync.dma_start(
                out=y_d[c, r_out0 : r_out0 + 112, :], in_=outb[0:112, :]
            )
            if t < NSTR - 1:
                nc.sync.dma_start(
                    out=y_d[c, r_out0 + 112 : r_out0 + MSTR, :],
                    in_=outb[112:MSTR, :],
                )
            del tiles[idx]

        # Two-deep software pipeline tuned for the in-order engine queues
        # AND the PE p-state ramp: loads+compacts run one stripe ahead; PE
        # executes stripe i-1's 8 late matmuls back-to-back with stripe i's
        # 4 early ones; ACT retires rsqrt_{i-1} (stale deps) while the PE
        # burst finishes, THEN s1sq_i; stt+stores of i-1 go last so the
        # store issue never blocks the ACT stream.
        NTOT = C * NSTR
        stage_load(0)
        for idx in range(NTOT):
            if idx + 1 < NTOT:
                stage_load(idx + 1)
            stage_vec(idx)
            if idx >= 1:
                stage_mm_late(idx - 1)
                stage_rsq(idx - 1)
            stage_mm_early(idx)
            stage_s1sq(idx)
            if idx >= 1:
                stage_fin(idx - 1)
        stage_mm_late(NTOT - 1)
        stage_rsq(NTOT - 1)
        stage_fin(NTOT - 1)

    nc.finalize()
    return nc


def _get_nc():
    if "nc" not in _CACHE:
        _CACHE["nc"] = _build_nc()
    return _CACHE["nc"]


def kernel(x: np.ndarray, _trace: bool = False, _tmpdir=None) -> np.ndarray:
    from concourse.bass_utils import run_bass_kernel_spmd

    assert x.shape == (NCORES, C, H, W), x.shape
    nc = _get_nc()
    bands, iden, corr = _const_mats()
    in_maps = [
        {
            "x": np.ascontiguousarray(x[i]).astype(np.float32, copy=False),
            "bands": bands,
            "iden": iden,
            "corr": corr,
        }
        for i in range(NCORES)
    ]
    res = run_bass_kernel_spmd(
        nc,
        in_maps,
        core_ids=list(range(NCORES)),
        trace=_trace,
        tmpdir=_tmpdir,
    )
    _CACHE["last_results"] = res
    out = np.empty((NCORES, C, H, W), np.float32)
    for i, r in enumerate(res.results):
        buf = r["y"]  # half-layout: cols [0:512]=even, [512:1024]=odd
        out[i, ..., 0::2] = buf[..., 0:512]
        out[i, ..., 1::2] = buf[..., 512:1024]
    return out


if __name__ == "__main__":
    rng = np.random.default_rng(0)
    x = rng.random((NCORES, C, H, W), dtype=np.float32)
    y = kernel(x)
    print(y.shape, y.dtype, float(np.abs(y).mean()))


# revision 13
# speedup vs baseline: 1.0641x; 1.0641x over previous
"""LocalContrastEnhancement v4: decimated-horizontal-scan Bass kernel, 8 trn2 cores.

out = (x - mean) / (sqrt(max(var, 1e-6)) + 1e-6), 15x15 zero-padded box.

Sharding: pure data parallel, 1 image (3,1024,1024) per NeuronCore.

v4 halves the DVE scan cost (the v1-v3 bottleneck: 2 scans x 2.17ns/elem
x 1031 cols) by folding columns into pairs and scanning a 7-PAIR window
(14-col sums at stride 2), then adding the 15th column inside the PE
band matmuls as an extra accumulated matmul over a compact fp16
even/odd column tile. Even and odd output columns become two 512-wide
phases sharing the same scan output.

Per stripe (K<=128 input rows, M=114 out rows):
  ACT: xe/xo = fp16(x[even/odd cols]), sqe/sqo = fp16((x-.5)^2[e/o])
       (strided reads, compact outputs -> every matmul operand is
       contiguous), s1sq = (S1~)^2 and rsqrt over merged 1024-wide psum.
  DVE: fold yx=xe+xo, ysq=sqe+sqo (fp16 2x), two 519-long scans
       (7-pair windows, centered via the scan initial), two 512-wide
       STTs writing interleaved columns of the group output tile.
  PE (12 matmuls/stripe, all 512-free fp16):
       PD[:,e] = -band*o1 - band*xo[corr] + iden*xe   (odd mirrored)
       P2[:,e] = 225*band*o2 + 225*band*sqo[corr] - I*s1sq
  Vertical pad rows are corrected via per-row constants folded into the
  ACT biases / STT scalar (raw-pad algebra: see corr vectors).

DMA queue rule (measured): a transfer lands on the largest divisor
<=16 of its PARTITION count many queues. 114-partition stores hit only
6 queues; so stores are split at partition 112 (=16*7 -> all 16
queues), and input loads likewise avoid non-16-divisible row counts.
"""

import numpy as np

C, H, W = 3, 1024, 1024
NCORES = 8
KS = 15
HALF = 7
XP = 8  # left pad cols in the xt row buffer
BX = XP + W + 8  # 1040
NP = 520  # compact even/odd tile length (image cols -8..1031)
YB = 527  # fold buffer: 7 left pad pairs + 520
NSC = 519  # scan output length; o1[s] = 14-col sum for out col pair j=s-7
MSTR = 114  # out rows per stripe (uniform; bottom stripe rows >=1024 trimmed)
NSTR = 9  # stripes per channel
GRP = 3  # stripes batched per output store

_CACHE = {}


def _stripes():
    """(r_in0, K, variant) per stripe; r_out0 = 114*t. variant: 0 top, 1 bottom, 2 interior."""
    out = []
    for t in range(NSTR):
        r_out0 = MSTR * t
        r_in0 = max(r_out0 - HALF, 0)
        r_in1 = min(r_out0 + MSTR - 1 + HALF, H - 1)
        k = r_in1 - r_in0 + 1
        v = 0 if t == 0 else (1 if t == NSTR - 1 else 2)
        out.append((r_in0, k, v))
    return out


def _const_mats():
    band = np.zeros((128, MSTR), dtype=np.float32)
    iden = np.zeros((128, MSTR), dtype=np.float32)
    for m in range(MSTR):
        band[m : m + KS, m] = 1.0
        iden[m + HALF, m] = 225.0
    band_top = np.zeros_like(band)
    band_top[0:121, :] = band[7:128, :]
    iden_top = np.zeros_like(iden)
    iden_top[0:121, :] = iden[7:128, :]
    negi = np.zeros((128, MSTR), dtype=np.float32)
    for m in range(MSTR):
        negi[m, m] = -1.0
    bands = np.stack(
        [-band, 225.0 * band, -band_top, 225.0 * band_top, negi], axis=1
    )  # [128, 5, 114] fp16
    idens = np.stack([iden, iden_top], axis=1).astype(np.float16)

    # Per-out-row vertical pad corrections (raw-pad algebra):
    #   s1sq bias   = -7.5 - 7n      (S1~true = -PD_ph1 - 7.5 - 7n)
    #   rsqrt bias  = 843.75 n       (225*S2~true = P2 + 843.75n)
    #   stt scalar  = 7n - 105       (num = PD + 7n - 105)
    m_idx = np.arange(128)
    n_top = np.maximum(0, HALF - m_idx).astype(np.float32)
    n_bot = np.maximum(0, m_idx - 104).astype(np.float32)
    corr = np.zeros((128, 3, 3), dtype=np.float32)
    for v, n in ((0, n_top), (1, n_bot), (2, np.zeros(128, np.float32))):
        corr[:, v, 0] = -7.5 - 7.0 * n
        corr[:, v, 1] = 843.75 * n
        corr[:, v, 2] = 7.0 * n - 105.0
    return bands.astype(np.float16), idens, corr


def _build_nc():
    import concourse.bass as bass
    import concourse.bacc as bacc
    import concourse.tile as tile
    from concourse import mybir
    import bass_rust as _bass_rust
    from concourse.hw_specs import get_activation_tables

    f32 = mybir.dt.float32
    fp16 = mybir.dt.float16
    Alu = mybir.AluOpType
    Act = mybir.ActivationFunctionType

    class _LceBacc(bacc.Bacc):
        """Pin act-table selection to the set holding Square+Copy+AbsRsqrt."""

        def insert_act_table_loads(self):
            tables = [
                (name, funcs if name == "abs_reciprocal_sqrt_and_small" else set())
                for name, funcs in get_activation_tables(self.m.arch).items()
            ]
            _bass_rust.insert_act_table_loads(self, tables)

    nc = _LceBacc(trn_type="TRN2", target_bir_lowering=False)
    x_d = nc.dram_tensor("x", [C, H, W], f32, kind="ExternalInput")
    bands_d = nc.dram_tensor("bands", [128, 5, MSTR], fp16, kind="ExternalInput")
    iden_d = nc.dram_tensor("iden", [128, 2, MSTR], fp16, kind="ExternalInput")
    corr_d = nc.dram_tensor("corr", [128, 3, 3], f32, kind="ExternalInput")
    y_d = nc.dram_tensor("y", [C, H, W], f32, kind="ExternalOutput")

    stripes = _stripes()

    from contextlib import ExitStack

    with tile.TileContext(nc) as tc, ExitStack() as ctx:
        singles = ctx.enter_context(tc.tile_pool(name="singles", bufs=1))
        io_pool = ctx.enter_context(tc.tile_pool(name="io", bufs=1))
        s1sq_p = ctx.enter_context(tc.tile_pool(name="s1sq", bufs=4))
        r_p = ctx.enter_context(tc.tile_pool(name="rts", bufs=4))
        out_p = ctx.enter_context(tc.tile_pool(name="outb", bufs=4))
        psd_p = ctx.enter_context(tc.tile_pool(name="psd", bufs=2, space="PSUM"))
        ps2_p = ctx.enter_context(tc.tile_pool(name="ps2", bufs=2, space="PSUM"))

        bands_t = singles.tile([128, 5, MSTR], fp16)
        iden_t = singles.tile([128, 2, MSTR], fp16)
        corr_t = singles.tile([128, 3, 3], f32)
        nc.sync.dma_start(out=bands_t[:, :, :], in_=bands_d[:, :, :])
        nc.sync.dma_start(out=iden_t[:, :, :], in_=iden_d[:, :, :])
        nc.sync.dma_start(out=corr_t[:, :, :], in_=corr_d[:, :, :])

        NBUF = 6
        xb = [io_pool.tile([128, BX], f32, tag=f"xb{i}", name=f"xb{i}") for i in range(NBUF)]
        xe = [io_pool.tile([128, NP], fp16, tag=f"xe{i}", name=f"xe{i}") for i in range(NBUF)]
        xo = [io_pool.tile([128, NP], fp16, tag=f"xo{i}", name=f"xo{i}") for i in range(NBUF)]
        sqe = [io_pool.tile([128, NP], fp16, tag=f"sqe{i}", name=f"sqe{i}") for i in range(NBUF)]
        sqo = [io_pool.tile([128, NP], fp16, tag=f"sqo{i}", name=f"sqo{i}") for i in range(NBUF)]
        yx = [io_pool.tile([128, YB], fp16, tag=f"yx{i}", name=f"yx{i}") for i in range(NBUF)]
        ysq = [io_pool.tile([128, YB], fp16, tag=f"ysq{i}", name=f"ysq{i}") for i in range(NBUF)]
        ob1 = [io_pool.tile([128, NSC], fp16, tag=f"ob1{i}", name=f"ob1{i}") for i in range(NBUF)]
        ob2 = [io_pool.tile([128, NSC], fp16, tag=f"ob2{i}", name=f"ob2{i}") for i in range(NBUF)]
        for i in range(NBUF):
            nc.vector.memset(xb[i][:, 0:XP], 0.0)
            nc.vector.memset(xb[i][:, XP + W : BX], 0.0)
            nc.vector.memset(yx[i][:, 0:7], 0.0)
            nc.vector.memset(ysq[i][:, 0:7], 0.5)

        neghalf = singles.tile([128, 1], f32)
        nc.vector.memset(neghalf[:, :], -0.5)
        # ACT warm-ups: absorb const-DMA / memset sync ticks outside the loop
        warm1 = singles.tile([128, 1], f32)
        warm2 = singles.tile([128, 1], f32)
        warm3 = singles.tile([128, 1], f32)
        warm4 = singles.tile([128, 1], f32)
        nc.scalar.activation(out=warm1[:, :], in_=corr_t[:, 0, 0:1], func=Act.Square)
        nc.scalar.activation(out=warm2[:, :], in_=iden_t[:, 0, 0:1], func=Act.Square)
        nc.scalar.activation(out=warm3[:, :], in_=neghalf[:, :], func=Act.Square)
        nc.scalar.activation(
            out=warm4[:, :], in_=warm3[:, :], func=Act.Abs_reciprocal_sqrt
        )

        def stage_load(idx):
            """DMA in + compact casts for stripe idx (hoisted one stripe
            ahead so the in-order ACT queue never stalls on s1sq deps)."""
            c, t = divmod(idx, NSTR)
            r_in0, K, vv = stripes[t]
            i5 = idx % NBUF
            xt = xb[i5]

            # split loads so partition counts divide by 16 (queue spread)
            if K == 128:
                nc.sync.dma_start(
                    out=xt[0:K, XP : XP + W],
                    in_=x_d[c, r_in0 : r_in0 + K, :],
                )
            else:
                nc.sync.dma_start(
                    out=xt[0:112, XP : XP + W],
                    in_=x_d[c, r_in0 : r_in0 + 112, :],
                )
                nc.sync.dma_start(
                    out=xt[112:K, XP : XP + W],
                    in_=x_d[c, r_in0 + 112 : r_in0 + K, :],
                )

            # compact fp16 even/odd tiles (strided ACT reads)
            nc.scalar.activation(
                out=xe[i5][0:K, :], in_=xt[0:K, 0:BX:2], func=Act.Copy
            )
            nc.scalar.activation(
                out=xo[i5][0:K, :], in_=xt[0:K, 1:BX:2], func=Act.Copy
            )
            nc.scalar.activation(
                out=sqe[i5][0:K, :],
                in_=xt[0:K, 0:BX:2],
                func=Act.Square,
                bias=neghalf[0:K, 0:1],
            )
            nc.scalar.activation(
                out=sqo[i5][0:K, :],
                in_=xt[0:K, 1:BX:2],
                func=Act.Square,
                bias=neghalf[0:K, 0:1],
            )

        def stage_vec(idx):
            """Pair folds + scans for stripe idx."""
            c, t = divmod(idx, NSTR)
            r_in0, K, vv = stripes[t]
            i5 = idx % NBUF
            # pair folds (fp16 2x) and 7-pair scans (centered via initial)
            nc.vector.tensor_tensor(
                out=yx[i5][0:K, 7:YB],
                in0=xe[i5][0:K, :],
                in1=xo[i5][0:K, :],
                op=Alu.add,
            )
            nc.vector.tensor_tensor(
                out=ysq[i5][0:K, 7:YB],
                in0=sqe[i5][0:K, :],
                in1=sqo[i5][0:K, :],
                op=Alu.add,
            )
            nc.vector.tensor_tensor_scan(
                out=ob1[i5][0:K, 0:NSC],
                data0=yx[i5][0:K, 7 : 7 + NSC],
                data1=yx[i5][0:K, 0:NSC],
                initial=-7.0,
                op0=Alu.add,
                op1=Alu.subtract,
            )
            nc.vector.tensor_tensor_scan(
                out=ob2[i5][0:K, 0:NSC],
                data0=ysq[i5][0:K, 7 : 7 + NSC],
                data1=ysq[i5][0:K, 0:NSC],
                initial=3.5,
                op0=Alu.add,
                op1=Alu.subtract,
            )

        tiles = {}

        def stage_mm_early(idx):
            """Phase-1 band matmuls (4) for stripe idx; allocates pd."""
            c, t = divmod(idx, NSTR)
            r_in0, K, vv = stripes[t]
            i5 = idx % NBUF
            bsel = 2 if vv == 0 else 0
            o1 = ob1[i5]
            pd = psd_p.tile([MSTR, W], f32, tag="pd", name="pd")
            tiles[idx] = {"pd": pd}
            nc.tensor.matmul(
                pd[0:MSTR, 0:512],
                bands_t[0:K, bsel, 0:MSTR],
                o1[0:K, 7 : 7 + 512],
                start=True,
                stop=False,
            )
            nc.tensor.matmul(
                pd[0:MSTR, 0:512],
                bands_t[0:K, bsel, 0:MSTR],
                xo[i5][0:K, 0:512],
                start=False,
                stop=False,
            )
            nc.tensor.matmul(
                pd[0:MSTR, 512:1024],
                bands_t[0:K, bsel, 0:MSTR],
                o1[0:K, 7 : 7 + 512],
                start=True,
                stop=False,
            )
            nc.tensor.matmul(
                pd[0:MSTR, 512:1024],
                bands_t[0:K, bsel, 0:MSTR],
                xe[i5][0:K, 8:520],
                start=False,
                stop=False,
            )

        def stage_s1sq(idx):
            """s1sq = (-PD + corr0)^2, fp16, mid-group psum read."""
            c, t = divmod(idx, NSTR)
            r_in0, K, vv = stripes[t]
            pd = tiles[idx]["pd"]
            s1sq = s1sq_p.tile([MSTR, W], fp16, tag="s1sq", name="s1sq")
            tiles[idx]["s1sq"] = s1sq
            nc.scalar.activation(
                out=s1sq[0:MSTR, :],
                in_=pd[0:MSTR, :],
                func=Act.Square,
                scale=-1.0,
                bias=corr_t[0:MSTR, vv, 0:1],
            )

        def stage_mm_late(idx):
            """Phase-2 iden matmuls (2) + P2 matmuls (6) for stripe idx."""
            c, t = divmod(idx, NSTR)
            r_in0, K, vv = stripes[t]
            i5 = idx % NBUF
            bsel = 2 if vv == 0 else 0
            isel = 1 if vv == 0 else 0
            o2 = ob2[i5]
            pd = tiles[idx]["pd"]
            s1sq = tiles[idx]["s1sq"]
            p2 = ps2_p.tile([MSTR, W], f32, tag="p2", name="p2")
            tiles[idx]["p2"] = p2
            nc.tensor.matmul(
                pd[0:MSTR, 0:512],
                iden_t[0:K, isel, 0:MSTR],
                xe[i5][0:K, 4:516],
                start=False,
                stop=True,
                skip_group_check=True,
            )
            nc.tensor.matmul(
                pd[0:MSTR, 512:1024],
                iden_t[0:K, isel, 0:MSTR],
                xo[i5][0:K, 4:516],
                start=False,
                stop=True,
                skip_group_check=True,
            )
            nc.tensor.matmul(
                p2[0:MSTR, 0:512],
                bands_t[0:K, bsel + 1, 0:MSTR],
                o2[0:K, 7 : 7 + 512],
                start=True,
                stop=False,
            )
            nc.tensor.matmul(
                p2[0:MSTR, 0:512],
                bands_t[0:K, bsel + 1, 0:MSTR],
                sqo[i5][0:K, 0:512],
                start=False,
                stop=False,
            )
            nc.tensor.matmul(
                p2[0:MSTR, 0:512],
                bands_t[0:MSTR, 4, 0:MSTR],
                s1sq[0:MSTR, 0:512],
                start=False,
                stop=True,
            )
            nc.tensor.matmul(
                p2[0:MSTR, 512:1024],
                bands_t[0:K, bsel + 1, 0:MSTR],
                o2[0:K, 7 : 7 + 512],
                start=True,
                stop=False,
            )
            nc.tensor.matmul(
                p2[0:MSTR, 512:1024],
                bands_t[0:K, bsel + 1, 0:MSTR],
                sqe[i5][0:K, 8:520],
                start=False,
                stop=False,
            )
            nc.tensor.matmul(
                p2[0:MSTR, 512:1024],
                bands_t[0:MSTR, 4, 0:MSTR],
                s1sq[0:MSTR, 512:1024],
                start=False,
                stop=True,
            )

        def stage_fin(idx):
            """rsqrt + final combine + stores for stripe idx."""
            c, t = divmod(idx, NSTR)
            r_in0, K, vv = stripes[t]
            r_out0 = MSTR * t
            pd = tiles[idx]["pd"]
            p2 = tiles[idx]["p2"]
            rts = r_p.tile([MSTR, W], f32, tag="rts", name="rts")
            nc.scalar.activation(
                out=rts[0:MSTR, :],
                in_=p2[0:MSTR, :],
                func=Act.Abs_reciprocal_sqrt,
                bias=corr_t[0:MSTR, vv, 1:2],
            )
            # out = (PD + corr2) * R in half-layout (cols [even|odd]);
            # python de-interleaves during unshard
            outb = out_p.tile([MSTR, W], f32, tag="outb", name="outb")
            nc.vector.scalar_tensor_tensor(
                out=outb[0:MSTR, 0:W],
                in0=pd[0:MSTR, 0:W],
                scalar=corr_t[0:MSTR, vv, 2:3],
                in1=rts[0:MSTR, 0:W],
                op0=Alu.add,
                op1=Alu.mult,
            )
            # stores: 112 partitions -> 16 queues; 2-row remainder apart
            nc.sync.dma_start(
                out=y_d[c, r_out0 : r_out0 + 112, :], in_=outb[0:112, :]
            )
            if t < NSTR - 1:
                nc.scalar.dma_start(
                    out=y_d[c, r_out0 + 112 : r_out0 + MSTR, :],
                    in_=outb[112:MSTR, :],
                )
            del tiles[idx]

        # Two-deep software pipeline tuned for the in-order engine queues
        # AND the PE p-state ramp: loads+compacts run one stripe ahead (ACT
        # never stalls on s1sq deps), and PE executes stripe i-1's 8 late
        # matmuls back-to-back with stripe i's 4 early ones as a single
        # 12-matmul burst (dependencies one iteration stale), keeping the
        # ramp warm. stt/rsqrt of stripe i-1 retire during iteration i.
        NTOT = C * NSTR
        stage_load(0)
        for idx in range(NTOT):
            if idx + 1 < NTOT:
                stage_load(idx + 1)
            stage_vec(idx)
            if idx >= 1:
                stage_mm_late(idx - 1)
            stage_mm_early(idx)
            stage_s1sq(idx)
            if idx >= 1:
                stage_fin(idx - 1)
        stage_mm_late(NTOT - 1)
        stage_fin(NTOT - 1)

    nc.finalize()
    return nc


def _get_nc():
    if "nc" not in _CACHE:
        _CACHE["nc"] = _build_nc()
    return _CACHE["nc"]


def kernel(x: np.ndarray, _trace: bool = False, _tmpdir=None) -> np.ndarray:
    from concourse.bass_utils import run_bass_kernel_spmd

    assert x.shape == (NCORES, C, H, W), x.shape
    nc = _get_nc()
    bands, iden, corr = _const_mats()
    in_maps = [
        {
            "x": np.ascontiguousarray(x[i]).astype(np.float32, copy=False),
            "bands": bands,
            "iden": iden,
            "corr": corr,
        }
        for i in range(NCORES)
    ]
    res = run_bass_kernel_spmd(
        nc,
        in_maps,
        core_ids=list(range(NCORES)),
        trace=_trace,
        tmpdir=_tmpdir,
    )
    _CACHE["last_results"] = res
    out = np.empty((NCORES, C, H, W), np.float32)
    for i, r in enumerate(res.results):
        buf = r["y"]  # half-layout: cols [0:512]=even, [512:1024]=odd
        out[i, ..., 0::2] = buf[..., 0:512]
        out[i, ..., 1::2] = buf[..., 512:1024]
    return out


if __name__ == "__main__":
    rng = np.random.default_rng(0)
    x = rng.random((NCORES, C, H, W), dtype=np.float32)
    y = kernel(x)
    print(y.shape, y.dtype, float(np.abs(y).mean()))
